# revision 28
# baseline (speedup 1.0000x reference)
"""Causal multi-head attention on 8 TRN2 NeuronCores.

Problem: x[4, 2048, 2048] @ Wq/Wk/Wv[2048, 2048] -> 16-head causal attention
(head_dim 128) -> out-proj Wo[2048, 2048] + b_out.

Sharding: 4-way head tensor-parallel x 2-way batch data-parallel.
Core c handles head group (c % 4) (4 heads = 512 cols of Wq/Wk/Wv, 512 rows
of Wo) and batch pair (c // 4). Each core emits a partial out-projection for
its 2 batches; the host sums the 4 partials per batch pair (the "all-reduce")
and adds the bias.

v2 design (vs the DRAM-staging baseline):
  - Host pre-casts x/weights to bf16 and pre-transposes x into chunk-major
    xT layout, so the kernel does zero on-device casts/transposes and every
    input DMA is one large contiguous transfer.
  - qT/kT/v and ctxT stay SBUF-resident for the whole kernel: no DRAM
    staging round-trips, P2/P3 read operands directly from SBUF.
  - Softmax normalization happens in P2: den row -> SBUF -> PE broadcast
    (ones-column matmul) -> DVE reciprocal on the [128,512] broadcast ->
    DVE multiply into ctxT. P3 is a pure out-projection.
  - Partial outputs leave the device in bf16 (host accumulates in f32).

Per-core pipeline (bf16 matmul operands, fp32 PSUM accumulation):
  P1: per 512-row chunk: load xT chunk, project to qT/kT [d, s] and
      v [s, d] straight into SBUF residents.
  P2: per (batch, head): scoresT[sk, sq] = kT.T @ qT, exp via ScalarE with
      1/sqrt(128) folded into the activation pre-scale (no max subtraction:
      |scores| <= ~5), causal mask via affine_select on diagonal tiles only,
      denominator via ones-vector matmul, ctxT accumulated in PSUM and
      normalized into SBUF on the way out.
  P3: out-proj y = ctxT.T @ Wo per 128-row tile; bf16 partial DMA'd out.
"""

import math

import numpy as np

P = 128
S = 2048          # sequence length
D = 2048          # model dim
NB = 2            # batches per core
SL = NB * S       # local rows (4096)
DL = 512          # local head dims (4 heads x 128)
HL = 4            # local heads
NI = D // P       # 16 i-tiles
SCHUNK = 512
NCHUNK = SL // SCHUNK  # 8
SCALE = 1.0 / math.sqrt(128.0)
N_CORES = 8

_CACHE = {}


def _split_multi_waits(nc):
    """This walrus build accepts at most ONE sync-wait per instruction
    (setupSyncWait: 'Too many sync wait commands'), but Tile emits up to
    ~3 waits per instruction and the kernel-tail drain carries one wait per
    outstanding semaphore. Hoist excess waits onto single-wait nops inserted
    immediately before the instruction on the same engine stream."""
    import bass_rust

    SyncInfo = bass_rust.SyncInfo
    n = 0
    for f in nc.m.functions:
        for b in f.blocks:
            out = []
            changed = False
            for inst in list(b.instructions):
                si = getattr(inst, "sync_info", None)
                if si is not None and si.on_wait and len(si.on_wait) > 1:
                    waits = list(si.on_wait)
                    for w in waits[:-1]:
                        n += 1
                        nop = bass_rust.InstNoOp(
                            name=f"waitsplit-{n}", ins=[], outs=[]
                        )
                        nop.engine = inst.engine
                        nop.sync_info = SyncInfo(on_wait=[w], on_update=[])
                        out.append(nop)
                    inst.sync_info = SyncInfo(
                        on_wait=[waits[-1]], on_update=list(si.on_update or [])
                    )
                    changed = True
                out.append(inst)
            if changed:
                b.instructions = out


def _build():
    import concourse.bass as bass
    import concourse.mybir as mybir
    import concourse.tile as tile

    f32 = mybir.dt.float32
    bf16 = mybir.dt.bfloat16

    nc = bass.Bass()
    xt_in = nc.declare_dram_parameter("xt", [NCHUNK * P, NI * SCHUNK], bf16, isOutput=False)
    wq_in = nc.declare_dram_parameter("wq", [P, NI * DL], bf16, isOutput=False)
    wk_in = nc.declare_dram_parameter("wk", [P, NI * DL], bf16, isOutput=False)
    wv_in = nc.declare_dram_parameter("wv", [P, NI * DL], bf16, isOutput=False)
    wo_in = nc.declare_dram_parameter("wo", [P, HL * D], bf16, isOutput=False)
    y_out = nc.declare_dram_parameter("y", [SL, D], bf16, isOutput=True)

    with tile.TileContext(nc) as tc:
        _emit(nc, tc, mybir, xt_in, wq_in, wk_in, wv_in, wo_in, y_out)
    _split_multi_waits(nc)
    return nc


def _emit(nc, tc, mybir, xt_in, wq_in, wk_in, wv_in, wo_in, y_out):
    from contextlib import ExitStack

    f32 = mybir.dt.float32
    bf16 = mybir.dt.bfloat16
    Exp = mybir.ActivationFunctionType.Exp

    ctx = ExitStack()
    with ctx:
        consts = ctx.enter_context(tc.tile_pool(name="consts", bufs=1))
        res_pool = ctx.enter_context(tc.tile_pool(name="res_pool", bufs=1))

        ones = consts.tile([P, 1], bf16, name="ones")
        nc.vector.memset(ones, 1.0)
        ones1 = consts.tile([1, P], bf16, name="ones1")
        nc.vector.memset(ones1, 1.0)

        qT_sb = res_pool.tile([P, HL, SL], bf16, name="qT_sb")
        kT_sb = res_pool.tile([P, HL, SL], bf16, name="kT_sb")
        v_sb = res_pool.tile([P, SL // P, DL], bf16, name="v_sb")

        # --- P1: project to qT/kT (d-major) and v (s-major), SBUF-resident ---
        NH = NI // 2  # 8 i-tiles per half
        HW = NH * DL  # flat half width
        with tc.tile_pool(name="wqkv", bufs=1) as wpool, \
             tc.tile_pool(name="xt_pool", bufs=2) as xt_pool, \
             tc.tile_pool(name="pbig1", bufs=2, space="PSUM") as pbig1:
            # wq and the first x chunk are split in halves across four DMA
            # queues so the very first matmuls (i=0..7 of the first q group)
            # start after ~1MB instead of ~4MB of DMA.
            wq_a = wpool.tile([P, NH, DL], bf16, name="wq_a")
            wq_b = wpool.tile([P, NH, DL], bf16, name="wq_b")
            wk_sb = wpool.tile([P, NI, DL], bf16, name="wk_sb")
            wv_sb = wpool.tile([P, NI, DL], bf16, name="wv_sb")
            x0_a = xt_pool.tile([P, NH, SCHUNK], bf16, name="x0_a", tag="x0a", bufs=1)
            x0_b = xt_pool.tile([P, NH, SCHUNK], bf16, name="x0_b", tag="x0b", bufs=1)
            # three usable DMA queues (sync/SP, scalar/ACT, gpsimd/SWDGE):
            # spread the startup-critical 4MB so the first q matmuls start
            # after ~1MB per queue instead of 4MB on one
            # queue plan (by observed queue spin-up: sync ~9us, scalar ~12us,
            # gpsimd/SWDGE ~15us): halves of the startup-critical tensors
            # alternate sync/scalar in need-order; wv (needed last) rides the
            # slow gpsimd queue.
            nc.sync.dma_start(
                out=x0_a.rearrange("p a s -> p (a s)"),
                in_=xt_in[:P, : NH * SCHUNK],
            )
            nc.scalar.dma_start(
                out=wq_a.rearrange("p a d -> p (a d)"), in_=wq_in[:, :HW]
            )
            nc.gpsimd.dma_start(
                out=wv_sb.rearrange("p a d -> p (a d)"), in_=wv_in[:, :]
            )
            nc.sync.dma_start(
                out=wq_b.rearrange("p a d -> p (a d)"), in_=wq_in[:, HW:]
            )
            nc.scalar.dma_start(
                out=x0_b.rearrange("p a s -> p (a s)"),
                in_=xt_in[:P, NH * SCHUNK :],
            )
            nc.sync.dma_start(
                out=wk_sb[:, :NH, :].rearrange("p a d -> p (a d)"),
                in_=wk_in[:, :HW],
            )
            nc.scalar.dma_start(
                out=wk_sb[:, NH:, :].rearrange("p a d -> p (a d)"),
                in_=wk_in[:, HW:],
            )

            def wq_sl(i, cols):
                t = wq_a if i < NH else wq_b
                return t[:, i % NH, cols]

            def load_chunk(ch):
                xT = xt_pool.tile([P, NI, SCHUNK], bf16, name="xT", tag="xT")
                nc.sync.dma_start(
                    out=xT.rearrange("p a s -> p (a s)"),
                    in_=xt_in[P * ch : P * (ch + 1), :],
                )
                return xT

            xT_next = load_chunk(1)
            for ch in range(NCHUNK):
                if ch == 0:
                    def x_sl(i, cols, a=x0_a, b=x0_b):
                        t = a if i < NH else b
                        return t[:, i % NH, cols]
                else:
                    xT = xT_next
                    if ch + 1 < NCHUNK:
                        xT_next = load_chunk(ch + 1)

                    def x_sl(i, cols, t=xT):
                        return t[:, i, cols]

                col0 = SCHUNK * ch
                # qT / kT: [d, s] layout; lhsT = W block, rhs = xT
                for w_sl, out_sb in (
                    (wq_sl, qT_sb),
                    (lambda i, cols: wk_sb[:, i, cols], kT_sb),
                ):
                    for hp in range(2):
                        pq = pbig1.tile([P, 1024], f32, name="pq", tag="pb")
                        for h2 in range(2):
                            h = 2 * hp + h2
                            for i in range(NI):
                                nc.tensor.matmul(
                                    pq[:, 512 * h2 : 512 * (h2 + 1)],
                                    lhsT=w_sl(i, slice(P * h, P * (h + 1))),
                                    rhs=x_sl(i, slice(None)),
                                    start=(i == 0),
                                    stop=(i == NI - 1),
                                )
                        nc.scalar.copy(
                            out=out_sb[:, 2 * hp : 2 * hp + 2, col0 : col0 + SCHUNK],
                            in_=pq.rearrange("p (a b) -> p a b", a=2),
                        )
                # v: [s, d] layout; lhsT = xT block, rhs = Wv
                for sp in range(2):
                    pv = pbig1.tile([P, 1024], f32, name="pv", tag="pb")
                    for s2 in range(2):
                        st = 2 * sp + s2
                        for i in range(NI):
                            nc.tensor.matmul(
                                pv[:, 512 * s2 : 512 * (s2 + 1)],
                                lhsT=x_sl(i, slice(P * st, P * (st + 1))),
                                rhs=wv_sb[:, i, :],
                                start=(i == 0),
                                stop=(i == NI - 1),
                            )
                    n0 = 4 * ch + 2 * sp
                    nc.scalar.copy(
                        out=v_sb[:, n0 : n0 + 2, :],
                        in_=pv.rearrange("p (a b) -> p a b", a=2),
                    )

        # --- P2: causal attention per (batch, head), all-SBUF operands ---
        wo_pool = ctx.enter_context(tc.tile_pool(name="wo_pool", bufs=1))
        cT_pool = ctx.enter_context(tc.tile_pool(name="cT_pool", bufs=1))
        att_pool = ctx.enter_context(tc.tile_pool(name="att_pool", bufs=2))
        out_pool = ctx.enter_context(tc.tile_pool(name="out_pool", bufs=3))
        dram = ctx.enter_context(tc.tile_pool(name="dram", bufs=1, space="DRAM"))
        pbig = ctx.enter_context(tc.tile_pool(name="pbig", bufs=2, space="PSUM"))
        psmall = ctx.enter_context(tc.tile_pool(name="psmall", bufs=4, space="PSUM"))

        # wo only feeds P3 — loading it here keeps P1's SBUF peak low
        wo_sb = wo_pool.tile([P, HL, D], bf16, name="wo_sb")
        nc.scalar.dma_start(
            out=wo_sb.rearrange("p a d -> p (a d)"), in_=wo_in[:, :]
        )
        # ctxT split per batch so P3 tiles for batch 0 (whole-tile deps) can
        # run while batch 1's attention is still in flight
        cT_b = [
            cT_pool.tile([P, HL, S], bf16, name=f"cT_b{b}") for b in range(NB)
        ]

        bh_list = [(b, h) for b in range(NB) for h in range(HL)]
        pend = [None]
        pend_pairs = []
        deferred = []  # tail_b thunks (broadcast + normalize), run 2 chunks late
        DEPTH = 3  # pend pipeline depth in score-pairs

        def flush_pend(drain=False):
            if pend[0] is not None:
                pend[0]()
                pend[0] = None
            while deferred and (drain or len(deferred) >= 3):
                deferred.pop(0)()

        def emit_p3_tile(b, t, on_dve):
            # one 128-row out-projection tile; during the P2(b=1) window the
            # PSUM->SBUF copies ride DVE so ScalarE stays free for exps
            col0 = P * t
            ysb = out_pool.tile([P, 2048], bf16, name="ysb", tag="ysb")
            for fp in range(2):
                py = pbig.tile([P, 1024], f32, name="py", tag="pb")
                for f2 in range(2):
                    f = 2 * fp + f2
                    for dt in range(HL):
                        nc.tensor.matmul(
                            py[:, 512 * f2 : 512 * (f2 + 1)],
                            lhsT=cT_b[b][:, dt, col0 : col0 + P],
                            rhs=wo_sb[:, dt, 512 * f : 512 * (f + 1)],
                            start=(dt == 0),
                            stop=(dt == HL - 1),
                        )
                dst = ysb[:, 1024 * fp : 1024 * (fp + 1)]
                if on_dve:
                    nc.vector.tensor_copy(dst, py)
                else:
                    nc.scalar.copy(out=dst, in_=py)
            nc.scalar.dma_start(out=y_out[S * b + col0 : S * b + col0 + P, :], in_=ysb)

        for bh_i, (b, h) in enumerate(bh_list):
            if b == 1 and h == 0:
                # push all batch-0 normalize tails into the stream so the
                # interleaved P3(b=0) tiles below have their inputs final
                flush_pend(drain=True)
            for c in range(S // SCHUNK):  # 4 sq-chunks
                if b == 1:
                    # interleave one batch-0 out-projection tile per chunk:
                    # independent PE work that absorbs the exp (ScalarE)
                    # latency which otherwise rate-limits P2
                    flush_pend()
                    emit_p3_tile(0, 4 * h + c, on_dve=True)
                qtc = qT_sb[:, h, S * b + SCHUNK * c : S * b + SCHUNK * (c + 1)]
                pctx = psmall.tile([P, 512], f32, name="pctx", tag="ps", bufs=2)
                pden = psmall.tile([P, 512], f32, name="pden", tag="ps", bufs=2)
                jmax = 4 * c + 4  # sk-tiles with sk_start <= sq_end

                def emit_av_group(items, pctx=pctx, pden=pden, b=b, h=h, jmax=jmax):
                    # all ctx matmuls back-to-back, then all den matmuls:
                    # consecutive same-PSUM-target matmuls avoid the
                    # ~90ns/bank-switch pipeline penalty.
                    for at2, _, j0, s0s in items:
                        for j2 in range(2):
                            j = j0 + j2
                            s0 = s0s[j2]
                            nc.tensor.matmul(
                                pctx[:, s0:],
                                lhsT=v_sb[:, (S // P) * b + j, P * h : P * (h + 1)],
                                rhs=at2[:, 512 * j2 + s0 : 512 * (j2 + 1)],
                                start=(j == 0),
                                stop=(j == jmax - 1),
                            )
                    for _, asum, j0, _ in items:
                        nc.tensor.matmul(
                            pden[:1, :],
                            lhsT=ones,
                            rhs=asum,
                            start=(j0 == 0),
                            stop=(j0 == jmax - 2),
                        )

                def emit_tail(pctx=pctx, pden=pden, b=b, h=h, c=c):
                    # tail_a (now): free PSUM fast — raw ctx to SBUF, den row
                    # to SBUF, then the DRAM-bounce reciprocal spread:
                    # [1,512] den is 512 serial lanes-1 elements on DVE
                    # (~3.4us), but spread to [128,4] it is ~0.2us.
                    ctu = att_pool.tile([P, 512], bf16, name="ctu", tag="ctu", bufs=6)
                    nc.vector.tensor_copy(ctu, pctx)
                    den_sb = att_pool.tile([1, 512], f32, name="den_sb", tag="den", bufs=4)
                    nc.vector.tensor_copy(den_sb, pden[:1, :])
                    dd = dram.tile([512], f32, name="dd", tag="dd", bufs=4)
                    nc.sync.dma_start(out=dd, in_=den_sb)
                    dsp = att_pool.tile([P, 4], f32, name="dsp", tag="dsp", bufs=4)
                    nc.sync.dma_start(out=dsp, in_=dd.rearrange("(p f) -> p f", p=P))
                    rsp = att_pool.tile([P, 4], f32, name="rsp", tag="rsp", bufs=4)
                    nc.vector.reciprocal(rsp, dsp)
                    rspb = att_pool.tile([P, 4], bf16, name="rspb", tag="rspb", bufs=4)
                    nc.vector.tensor_copy(rspb, rsp)
                    rd = dram.tile([512], bf16, name="rd", tag="rd", bufs=4)
                    nc.sync.dma_start(out=rd.rearrange("(p f) -> p f", p=P), in_=rspb)
                    rrow = att_pool.tile([1, 512], bf16, name="rrow", tag="rrow", bufs=5)
                    nc.sync.dma_start(out=rrow, in_=rd.rearrange("s -> () s"))

                    def tail_b(ctu=ctu, rrow=rrow, b=b, h=h, c=c):
                        # chunks later: PE-broadcast the reciprocals and
                        # normalize into the ctxT resident.
                        pbc = psmall.tile([P, 512], f32, name="pbc", tag="pbc", bufs=2)
                        nc.tensor.matmul(pbc, lhsT=ones1, rhs=rrow, start=True, stop=True)
                        nc.vector.tensor_mul(
                            cT_b[b][:, h, SCHUNK * c : SCHUNK * (c + 1)],
                            ctu,
                            pbc,
                        )

                    deferred.append(tail_b)

                for jp in range(jmax // 2):
                    j0 = 2 * jp
                    ps2 = pbig.tile([P, 1024], f32, name="ps2", tag="pb")
                    # causal trim: diagonal sk-tile j (= 4c+m, m>0) only
                    # needs sq columns >= 128m — skip the fully-masked left
                    # part of the scores stream and the exp
                    s0s = []
                    for j2 in range(2):
                        j = j0 + j2
                        m = j - 4 * c
                        s0 = P * m if m > 0 else 0
                        s0s.append(s0)
                        nc.tensor.matmul(
                            ps2[:, 512 * j2 + s0 : 512 * (j2 + 1)],
                            lhsT=kT_sb[:, h, S * b + P * j : S * b + P * (j + 1)],
                            rhs=qtc[:, s0:],
                            start=True,
                            stop=True,
                        )
                    at2 = att_pool.tile([P, 1024], bf16, name="at2", tag="at2", bufs=7)
                    if s0s == [0, 0]:
                        nc.scalar.activation(at2, ps2, Exp, scale=SCALE)
                    else:
                        for j2 in range(2):
                            sl = slice(512 * j2 + s0s[j2], 512 * (j2 + 1))
                            nc.scalar.activation(at2[:, sl], ps2[:, sl], Exp, scale=SCALE)
                    if j0 >= 4 * c:  # diagonal pair: zero sk > sq
                        nc.gpsimd.affine_select(
                            out=at2.rearrange("p (a b) -> p a b", a=2),
                            in_=at2.rearrange("p (a b) -> p a b", a=2),
                            compare_op=mybir.AluOpType.is_ge,
                            fill=0.0,
                            base=(0 if j0 == 4 * c else -256),
                            channel_multiplier=-1,
                            pattern=[[-P, 2], [1, 512]],
                        )
                    # pair-sum on DVE so each den matmul covers 2 sk-tiles
                    asum = att_pool.tile([P, 512], bf16, name="asum", tag="asum", bufs=7)
                    nc.vector.tensor_add(asum, at2[:, :512], at2[:, 512:])
                    flush_pend()
                    pend_pairs.append((at2, asum, j0, s0s))
                    is_last = jp + 1 == jmax // 2
                    if len(pend_pairs) == DEPTH or is_last:
                        items = list(pend_pairs)
                        pend_pairs.clear()

                        def pend_fn(items=items, emit=emit_av_group,
                                    tail=(emit_tail if is_last else None)):
                            emit(items)
                            if tail is not None:
                                tail()

                        pend[0] = pend_fn
        flush_pend(drain=True)

        # --- P3 remainder: batch-1 out-projection ---
        for t in range(S // P):
            emit_p3_tile(1, t, on_dve=False)


def _get_nc():
    if "nc" not in _CACHE:
        _CACHE["nc"] = _build()
    return _CACHE["nc"]


def _host_prep(inputs):
    import ml_dtypes

    bf = ml_dtypes.bfloat16
    x = np.asarray(inputs["x"], dtype=np.float32)
    wq = np.asarray(inputs["W_query"], dtype=np.float32)
    wk = np.asarray(inputs["W_key"], dtype=np.float32)
    wv = np.asarray(inputs["W_value"], dtype=np.float32)
    wo = np.asarray(inputs["W_out"], dtype=np.float32)

    xbf = x.astype(bf)
    xts = []
    for pair in range(2):
        xp = xbf[2 * pair : 2 * pair + 2]                      # [2, S, D]
        xT = np.ascontiguousarray(xp.transpose(2, 0, 1)).reshape(D, SL)
        xts.append(
            np.ascontiguousarray(
                xT.reshape(NI, P, NCHUNK, SCHUNK).transpose(2, 1, 0, 3)
            ).reshape(NCHUNK * P, NI * SCHUNK)
        )

    def colw(w, hg):
        return np.ascontiguousarray(
            w[:, DL * hg : DL * (hg + 1)].astype(bf).reshape(NI, P, DL).transpose(1, 0, 2)
        ).reshape(P, NI * DL)

    in_maps = []
    for c in range(N_CORES):
        pair, hg = c // 4, c % 4
        wo_h = np.ascontiguousarray(
            wo[DL * hg : DL * (hg + 1), :].astype(bf).reshape(HL, P, D).transpose(1, 0, 2)
        ).reshape(P, HL * D)
        in_maps.append(
            {
                "xt": xts[pair],
                "wq": colw(wq, hg),
                "wk": colw(wk, hg),
                "wv": colw(wv, hg),
                "wo": wo_h,
            }
        )
    return in_maps


def _run(inputs, trace=False):
    from concourse.bass_utils import run_bass_kernel_spmd

    in_maps = _host_prep(inputs)
    b_out = np.asarray(inputs["b_out"], dtype=np.float32)

    nc = _get_nc()
    res = run_bass_kernel_spmd(nc, in_maps, core_ids=list(range(N_CORES)), trace=trace)

    y = np.zeros((2, SL, D), dtype=np.float32)
    for c in range(N_CORES):
        y[c // 4] += res.results[c]["y"].astype(np.float32)
    y += b_out[None, None, :]
    out = y.reshape(4, S, D)
    return out, res


def kernel(**inputs) -> np.ndarray:
    out, _ = _run(inputs, trace=False)
    return out


# revision 33
# speedup vs baseline: 1.0508x; 1.0508x over previous
"""Causal multi-head attention on 8 TRN2 NeuronCores.

Problem: x[4, 2048, 2048] @ Wq/Wk/Wv[2048, 2048] -> 16-head causal attention
(head_dim 128) -> out-proj Wo[2048, 2048] + b_out.

Sharding: 4-way head tensor-parallel x 2-way batch data-parallel.
Core c handles head group (c % 4) (4 heads = 512 cols of Wq/Wk/Wv, 512 rows
of Wo) and batch pair (c // 4). Each core emits a partial out-projection for
its 2 batches; the host sums the 4 partials per batch pair (the "all-reduce")
and adds the bias.

v2 design (vs the DRAM-staging baseline):
  - Host pre-casts x/weights to bf16 and pre-transposes x into chunk-major
    xT layout, so the kernel does zero on-device casts/transposes and every
    input DMA is one large contiguous transfer.
  - qT/kT/v and ctxT stay SBUF-resident for the whole kernel: no DRAM
    staging round-trips, P2/P3 read operands directly from SBUF.
  - Softmax normalization happens in P2: den row -> SBUF -> PE broadcast
    (ones-column matmul) -> DVE reciprocal on the [128,512] broadcast ->
    DVE multiply into ctxT. P3 is a pure out-projection.
  - Partial outputs leave the device in bf16 (host accumulates in f32).

Per-core pipeline (bf16 matmul operands, fp32 PSUM accumulation):
  P1: per 512-row chunk: load xT chunk, project to qT/kT [d, s] and
      v [s, d] straight into SBUF residents.
  P2: per (batch, head): scoresT[sk, sq] = kT.T @ qT, exp via ScalarE with
      1/sqrt(128) folded into the activation pre-scale (no max subtraction:
      |scores| <= ~5), causal mask via affine_select on diagonal tiles only,
      denominator via ones-vector matmul, ctxT accumulated in PSUM and
      normalized into SBUF on the way out.
  P3: out-proj y = ctxT.T @ Wo per 128-row tile; bf16 partial DMA'd out.
"""

import math

import numpy as np

P = 128
S = 2048          # sequence length
D = 2048          # model dim
NB = 2            # batches per core
SL = NB * S       # local rows (4096)
DL = 512          # local head dims (4 heads x 128)
HL = 4            # local heads
NI = D // P       # 16 i-tiles
SCHUNK = 512
NCHUNK = SL // SCHUNK  # 8
SCALE = 1.0 / math.sqrt(128.0)
N_CORES = 8

_CACHE = {}


def _split_multi_waits(nc):
    """This walrus build accepts at most ONE sync-wait per instruction
    (setupSyncWait: 'Too many sync wait commands'), but Tile emits up to
    ~3 waits per instruction and the kernel-tail drain carries one wait per
    outstanding semaphore. Hoist excess waits onto single-wait nops inserted
    immediately before the instruction on the same engine stream."""
    import bass_rust

    SyncInfo = bass_rust.SyncInfo
    n = 0
    for f in nc.m.functions:
        for b in f.blocks:
            out = []
            changed = False
            for inst in list(b.instructions):
                si = getattr(inst, "sync_info", None)
                if si is not None and si.on_wait and len(si.on_wait) > 1:
                    waits = list(si.on_wait)
                    for w in waits[:-1]:
                        n += 1
                        nop = bass_rust.InstNoOp(
                            name=f"waitsplit-{n}", ins=[], outs=[]
                        )
                        nop.engine = inst.engine
                        nop.sync_info = SyncInfo(on_wait=[w], on_update=[])
                        out.append(nop)
                    inst.sync_info = SyncInfo(
                        on_wait=[waits[-1]], on_update=list(si.on_update or [])
                    )
                    changed = True
                out.append(inst)
            if changed:
                b.instructions = out


def _build():
    import concourse.bass as bass
    import concourse.mybir as mybir
    import concourse.tile as tile

    f32 = mybir.dt.float32
    bf16 = mybir.dt.bfloat16

    nc = bass.Bass()
    xt_in = nc.declare_dram_parameter("xt", [NCHUNK * P, NI * SCHUNK], bf16, isOutput=False)
    wq_in = nc.declare_dram_parameter("wq", [P, NI * DL], bf16, isOutput=False)
    wk_in = nc.declare_dram_parameter("wk", [P, NI * DL], bf16, isOutput=False)
    wv_in = nc.declare_dram_parameter("wv", [P, NI * DL], bf16, isOutput=False)
    wo_in = nc.declare_dram_parameter("wo", [P, HL * D], bf16, isOutput=False)
    y_out = nc.declare_dram_parameter("y", [SL, D], bf16, isOutput=True)

    with tile.TileContext(nc) as tc:
        _emit(nc, tc, mybir, xt_in, wq_in, wk_in, wv_in, wo_in, y_out)
    _split_multi_waits(nc)
    return nc


def _emit(nc, tc, mybir, xt_in, wq_in, wk_in, wv_in, wo_in, y_out):
    from contextlib import ExitStack

    f32 = mybir.dt.float32
    bf16 = mybir.dt.bfloat16
    Exp = mybir.ActivationFunctionType.Exp

    ctx = ExitStack()
    with ctx:
        consts = ctx.enter_context(tc.tile_pool(name="consts", bufs=1))
        res_pool = ctx.enter_context(tc.tile_pool(name="res_pool", bufs=1))

        ones = consts.tile([P, 1], bf16, name="ones")
        nc.vector.memset(ones, 1.0)
        ones1 = consts.tile([1, P], bf16, name="ones1")
        nc.vector.memset(ones1, 1.0)

        qT_sb = res_pool.tile([P, HL, SL], bf16, name="qT_sb")
        kT_sb = res_pool.tile([P, HL, SL], bf16, name="kT_sb")
        v_sb = res_pool.tile([P, SL // P, DL], bf16, name="v_sb")

        # --- P1: project to qT/kT (d-major) and v (s-major), SBUF-resident ---
        NH = NI // 2  # 8 i-tiles per half
        HW = NH * DL  # flat half width
        with tc.tile_pool(name="wqkv", bufs=1) as wpool, \
             tc.tile_pool(name="xt_pool", bufs=2) as xt_pool, \
             tc.tile_pool(name="pbig1", bufs=2, space="PSUM") as pbig1:
            # wq and the first x chunk are split in halves across four DMA
            # queues so the very first matmuls (i=0..7 of the first q group)
            # start after ~1MB instead of ~4MB of DMA.
            wq_a = wpool.tile([P, NH, DL], bf16, name="wq_a")
            wq_b = wpool.tile([P, NH, DL], bf16, name="wq_b")
            wk_sb = wpool.tile([P, NI, DL], bf16, name="wk_sb")
            wv_sb = wpool.tile([P, NI, DL], bf16, name="wv_sb")
            x0_a = xt_pool.tile([P, NH, SCHUNK], bf16, name="x0_a", tag="x0a", bufs=1)
            x0_b = xt_pool.tile([P, NH, SCHUNK], bf16, name="x0_b", tag="x0b", bufs=1)
            # three usable DMA queues (sync/SP, scalar/ACT, gpsimd/SWDGE):
            # spread the startup-critical 4MB so the first q matmuls start
            # after ~1MB per queue instead of 4MB on one
            # queue plan (by observed queue spin-up: sync ~9us, scalar ~12us,
            # gpsimd/SWDGE ~15us): halves of the startup-critical tensors
            # alternate sync/scalar in need-order; wv (needed last) rides the
            # slow gpsimd queue.
            nc.sync.dma_start(
                out=x0_a.rearrange("p a s -> p (a s)"),
                in_=xt_in[:P, : NH * SCHUNK],
            )
            nc.scalar.dma_start(
                out=wq_a.rearrange("p a d -> p (a d)"), in_=wq_in[:, :HW]
            )
            nc.gpsimd.dma_start(
                out=wv_sb.rearrange("p a d -> p (a d)"), in_=wv_in[:, :]
            )
            nc.sync.dma_start(
                out=wq_b.rearrange("p a d -> p (a d)"), in_=wq_in[:, HW:]
            )
            nc.scalar.dma_start(
                out=x0_b.rearrange("p a s -> p (a s)"),
                in_=xt_in[:P, NH * SCHUNK :],
            )
            nc.sync.dma_start(
                out=wk_sb[:, :NH, :].rearrange("p a d -> p (a d)"),
                in_=wk_in[:, :HW],
            )
            nc.scalar.dma_start(
                out=wk_sb[:, NH:, :].rearrange("p a d -> p (a d)"),
                in_=wk_in[:, HW:],
            )

            def wq_sl(i, cols):
                t = wq_a if i < NH else wq_b
                return t[:, i % NH, cols]

            def load_chunk(ch):
                xT = xt_pool.tile([P, NI, SCHUNK], bf16, name="xT", tag="xT")
                nc.sync.dma_start(
                    out=xT.rearrange("p a s -> p (a s)"),
                    in_=xt_in[P * ch : P * (ch + 1), :],
                )
                return xT

            xT_next = load_chunk(1)
            for ch in range(NCHUNK):
                if ch == 0:
                    def x_sl(i, cols, a=x0_a, b=x0_b):
                        t = a if i < NH else b
                        return t[:, i % NH, cols]
                else:
                    xT = xT_next
                    if ch + 1 < NCHUNK:
                        xT_next = load_chunk(ch + 1)

                    def x_sl(i, cols, t=xT):
                        return t[:, i, cols]

                col0 = SCHUNK * ch
                # qT / kT: [d, s] layout; lhsT = W block, rhs = xT
                for w_sl, out_sb in (
                    (wq_sl, qT_sb),
                    (lambda i, cols: wk_sb[:, i, cols], kT_sb),
                ):
                    for hp in range(2):
                        pq = pbig1.tile([P, 1024], f32, name="pq", tag="pb")
                        for h2 in range(2):
                            h = 2 * hp + h2
                            for i in range(NI):
                                nc.tensor.matmul(
                                    pq[:, 512 * h2 : 512 * (h2 + 1)],
                                    lhsT=w_sl(i, slice(P * h, P * (h + 1))),
                                    rhs=x_sl(i, slice(None)),
                                    start=(i == 0),
                                    stop=(i == NI - 1),
                                )
                        nc.scalar.copy(
                            out=out_sb[:, 2 * hp : 2 * hp + 2, col0 : col0 + SCHUNK],
                            in_=pq.rearrange("p (a b) -> p a b", a=2),
                        )
                # v: [s, d] layout; lhsT = xT block, rhs = Wv
                for sp in range(2):
                    pv = pbig1.tile([P, 1024], f32, name="pv", tag="pb")
                    for s2 in range(2):
                        st = 2 * sp + s2
                        for i in range(NI):
                            nc.tensor.matmul(
                                pv[:, 512 * s2 : 512 * (s2 + 1)],
                                lhsT=x_sl(i, slice(P * st, P * (st + 1))),
                                rhs=wv_sb[:, i, :],
                                start=(i == 0),
                                stop=(i == NI - 1),
                            )
                    n0 = 4 * ch + 2 * sp
                    nc.scalar.copy(
                        out=v_sb[:, n0 : n0 + 2, :],
                        in_=pv.rearrange("p (a b) -> p a b", a=2),
                    )

        # --- P2: causal attention per (batch, head), all-SBUF operands ---
        wo_pool = ctx.enter_context(tc.tile_pool(name="wo_pool", bufs=1))
        cT_pool = ctx.enter_context(tc.tile_pool(name="cT_pool", bufs=1))
        att_pool = ctx.enter_context(tc.tile_pool(name="att_pool", bufs=2))
        out_pool = ctx.enter_context(tc.tile_pool(name="out_pool", bufs=3))
        dram = ctx.enter_context(tc.tile_pool(name="dram", bufs=1, space="DRAM"))
        pbig = ctx.enter_context(tc.tile_pool(name="pbig", bufs=2, space="PSUM"))
        psmall = ctx.enter_context(tc.tile_pool(name="psmall", bufs=4, space="PSUM"))

        # wo only feeds P3 — loading it here keeps P1's SBUF peak low
        wo_sb = wo_pool.tile([P, HL, D], bf16, name="wo_sb")
        nc.scalar.dma_start(
            out=wo_sb.rearrange("p a d -> p (a d)"), in_=wo_in[:, :]
        )
        # ctxT split per batch so P3 tiles for batch 0 (whole-tile deps) can
        # run while batch 1's attention is still in flight
        cT_b = [
            cT_pool.tile([P, HL, S], bf16, name=f"cT_b{b}") for b in range(NB)
        ]

        bh_list = [(b, h) for b in range(NB) for h in range(HL)]
        pend = [None]
        pend_pairs = []
        deferred = []  # tail_b thunks (broadcast + normalize), run 2 chunks late
        p3q = []  # interleaved P3 half-tile thunks, one per flush point
        DEPTH = 3  # pend pipeline depth in score-pairs

        def flush_pend(drain=False):
            if pend[0] is not None:
                pend[0]()
                pend[0] = None
            while deferred and (drain or len(deferred) >= 3):
                deferred.pop(0)()
            if p3q:
                p3q.pop(0)()
                while drain and p3q:
                    p3q.pop(0)()

        def emit_p3_tile(b, t, on_dve):
            # one 128-row out-projection tile; during the P2(b=1) window the
            # PSUM->SBUF copies ride DVE so ScalarE stays free for exps
            col0 = P * t
            ysb = out_pool.tile([P, 2048], bf16, name="ysb", tag="ysb")
            for fp in range(2):
                py = pbig.tile([P, 1024], f32, name="py", tag="pb")
                for f2 in range(2):
                    f = 2 * fp + f2
                    for dt in range(HL):
                        nc.tensor.matmul(
                            py[:, 512 * f2 : 512 * (f2 + 1)],
                            lhsT=cT_b[b][:, dt, col0 : col0 + P],
                            rhs=wo_sb[:, dt, 512 * f : 512 * (f + 1)],
                            start=(dt == 0),
                            stop=(dt == HL - 1),
                        )
                dst = ysb[:, 1024 * fp : 1024 * (fp + 1)]
                if on_dve:
                    nc.vector.tensor_copy(dst, py)
                else:
                    nc.scalar.copy(out=dst, in_=py)
            nc.scalar.dma_start(out=y_out[S * b + col0 : S * b + col0 + P, :], in_=ysb)

        def queue_p3_tile(b, t):
            # interleaved variant: four quarter-tiles on a dedicated single
            # PSUM bank, released at consecutive flush points so they fill PE
            # bubbles without touching the scores' PSUM rotation
            col0 = P * t
            ysb = out_pool.tile([P, 2048], bf16, name="ysb", tag="ysb")

            def quarter(f, ysb=ysb, col0=col0, b=b):
                py = psmall.tile([P, 512], f32, name="py5", tag="py", bufs=1)
                for dt in range(HL):
                    nc.tensor.matmul(
                        py,
                        lhsT=cT_b[b][:, dt, col0 : col0 + P],
                        rhs=wo_sb[:, dt, 512 * f : 512 * (f + 1)],
                        start=(dt == 0),
                        stop=(dt == HL - 1),
                    )
                nc.vector.tensor_copy(ysb[:, 512 * f : 512 * (f + 1)], py)
                if f == 3:
                    nc.scalar.dma_start(
                        out=y_out[S * b + col0 : S * b + col0 + P, :], in_=ysb
                    )

            for f in range(4):
                p3q.append(lambda f=f: quarter(f))

        for bh_i, (b, h) in enumerate(bh_list):
            if b == 1 and h == 0:
                # push all batch-0 normalize tails into the stream so the
                # interleaved P3(b=0) tiles below have their inputs final
                flush_pend(drain=True)
            for c in range(S // SCHUNK):  # 4 sq-chunks
                if b == 1:
                    # interleave one batch-0 out-projection tile per chunk:
                    # independent PE work that absorbs the exp (ScalarE)
                    # latency which otherwise rate-limits P2
                    queue_p3_tile(0, 4 * h + c)
                qtc = qT_sb[:, h, S * b + SCHUNK * c : S * b + SCHUNK * (c + 1)]
                pctx = psmall.tile([P, 512], f32, name="pctx", tag="ps", bufs=2)
                pden = psmall.tile([P, 512], f32, name="pden", tag="ps", bufs=2)
                jmax = 4 * c + 4  # sk-tiles with sk_start <= sq_end

                def emit_av_group(items, pctx=pctx, pden=pden, b=b, h=h, jmax=jmax):
                    # all ctx matmuls back-to-back, then all den matmuls:
                    # consecutive same-PSUM-target matmuls avoid the
                    # ~90ns/bank-switch pipeline penalty.
                    for at2, _, j0, s0s in items:
                        for j2 in range(2):
                            j = j0 + j2
                            s0 = s0s[j2]
                            nc.tensor.matmul(
                                pctx[:, s0:],
                                lhsT=v_sb[:, (S // P) * b + j, P * h : P * (h + 1)],
                                rhs=at2[:, 512 * j2 + s0 : 512 * (j2 + 1)],
                                start=(j == 0),
                                stop=(j == jmax - 1),
                            )
                    for _, asum, j0, _ in items:
                        nc.tensor.matmul(
                            pden[:1, :],
                            lhsT=ones,
                            rhs=asum,
                            start=(j0 == 0),
                            stop=(j0 == jmax - 2),
                        )

                def emit_tail(pctx=pctx, pden=pden, b=b, h=h, c=c):
                    # tail_a (now): free PSUM fast — raw ctx to SBUF, den row
                    # to SBUF, then the DRAM-bounce reciprocal spread:
                    # [1,512] den is 512 serial lanes-1 elements on DVE
                    # (~3.4us), but spread to [128,4] it is ~0.2us.
                    ctu = att_pool.tile([P, 512], bf16, name="ctu", tag="ctu", bufs=6)
                    nc.vector.tensor_copy(ctu, pctx)
                    den_sb = att_pool.tile([1, 512], f32, name="den_sb", tag="den", bufs=4)
                    nc.vector.tensor_copy(den_sb, pden[:1, :])
                    dd = dram.tile([512], f32, name="dd", tag="dd", bufs=4)
                    nc.sync.dma_start(out=dd, in_=den_sb)
                    dsp = att_pool.tile([P, 4], f32, name="dsp", tag="dsp", bufs=4)
                    nc.sync.dma_start(out=dsp, in_=dd.rearrange("(p f) -> p f", p=P))
                    rsp = att_pool.tile([P, 4], f32, name="rsp", tag="rsp", bufs=4)
                    nc.vector.reciprocal(rsp, dsp)
                    rspb = att_pool.tile([P, 4], bf16, name="rspb", tag="rspb", bufs=4)
                    nc.vector.tensor_copy(rspb, rsp)
                    rd = dram.tile([512], bf16, name="rd", tag="rd", bufs=4)
                    nc.sync.dma_start(out=rd.rearrange("(p f) -> p f", p=P), in_=rspb)
                    rrow = att_pool.tile([1, 512], bf16, name="rrow", tag="rrow", bufs=5)
                    nc.sync.dma_start(out=rrow, in_=rd.rearrange("s -> () s"))

                    def tail_b(ctu=ctu, rrow=rrow, b=b, h=h, c=c):
                        # chunks later: PE-broadcast the reciprocals and
                        # normalize into the ctxT resident.
                        pbc = psmall.tile([P, 512], f32, name="pbc", tag="pbc", bufs=1)
                        nc.tensor.matmul(pbc, lhsT=ones1, rhs=rrow, start=True, stop=True)
                        nc.vector.tensor_mul(
                            cT_b[b][:, h, SCHUNK * c : SCHUNK * (c + 1)],
                            ctu,
                            pbc,
                        )

                    deferred.append(tail_b)

                for jp in range(jmax // 2):
                    j0 = 2 * jp
                    ps2 = pbig.tile([P, 1024], f32, name="ps2", tag="pb")
                    # causal trim: diagonal sk-tile j (= 4c+m, m>0) only
                    # needs sq columns >= 128m — skip the fully-masked left
                    # part of the scores stream and the exp
                    s0s = []
                    for j2 in range(2):
                        j = j0 + j2
                        m = j - 4 * c
                        s0 = P * m if m > 0 else 0
                        s0s.append(s0)
                        nc.tensor.matmul(
                            ps2[:, 512 * j2 + s0 : 512 * (j2 + 1)],
                            lhsT=kT_sb[:, h, S * b + P * j : S * b + P * (j + 1)],
                            rhs=qtc[:, s0:],
                            start=True,
                            stop=True,
                        )
                    at2 = att_pool.tile([P, 1024], bf16, name="at2", tag="at2", bufs=7)
                    if s0s == [0, 0]:
                        nc.scalar.activation(at2, ps2, Exp, scale=SCALE)
                    else:
                        for j2 in range(2):
                            sl = slice(512 * j2 + s0s[j2], 512 * (j2 + 1))
                            nc.scalar.activation(at2[:, sl], ps2[:, sl], Exp, scale=SCALE)
                    if j0 >= 4 * c:  # diagonal pair: zero sk > sq
                        nc.gpsimd.affine_select(
                            out=at2.rearrange("p (a b) -> p a b", a=2),
                            in_=at2.rearrange("p (a b) -> p a b", a=2),
                            compare_op=mybir.AluOpType.is_ge,
                            fill=0.0,
                            base=(0 if j0 == 4 * c else -256),
                            channel_multiplier=-1,
                            pattern=[[-P, 2], [1, 512]],
                        )
                    # pair-sum on DVE so each den matmul covers 2 sk-tiles
                    asum = att_pool.tile([P, 512], bf16, name="asum", tag="asum", bufs=7)
                    nc.vector.tensor_add(asum, at2[:, :512], at2[:, 512:])
                    flush_pend()
                    pend_pairs.append((at2, asum, j0, s0s))
                    is_last = jp + 1 == jmax // 2
                    if len(pend_pairs) == DEPTH or is_last:
                        items = list(pend_pairs)
                        pend_pairs.clear()

                        def pend_fn(items=items, emit=emit_av_group,
                                    tail=(emit_tail if is_last else None)):
                            emit(items)
                            if tail is not None:
                                tail()

                        pend[0] = pend_fn
        flush_pend(drain=True)

        # --- P3 remainder: batch-1 out-projection ---
        for t in range(S // P):
            emit_p3_tile(1, t, on_dve=False)


def _get_nc():
    if "nc" not in _CACHE:
        _CACHE["nc"] = _build()
    return _CACHE["nc"]


def _host_prep(inputs):
    import ml_dtypes

    bf = ml_dtypes.bfloat16
    x = np.asarray(inputs["x"], dtype=np.float32)
    wq = np.asarray(inputs["W_query"], dtype=np.float32)
    wk = np.asarray(inputs["W_key"], dtype=np.float32)
    wv = np.asarray(inputs["W_value"], dtype=np.float32)
    wo = np.asarray(inputs["W_out"], dtype=np.float32)

    xbf = x.astype(bf)
    xts = []
    for pair in range(2):
        xp = xbf[2 * pair : 2 * pair + 2]                      # [2, S, D]
        xT = np.ascontiguousarray(xp.transpose(2, 0, 1)).reshape(D, SL)
        xts.append(
            np.ascontiguousarray(
                xT.reshape(NI, P, NCHUNK, SCHUNK).transpose(2, 1, 0, 3)
            ).reshape(NCHUNK * P, NI * SCHUNK)
        )

    def colw(w, hg):
        return np.ascontiguousarray(
            w[:, DL * hg : DL * (hg + 1)].astype(bf).reshape(NI, P, DL).transpose(1, 0, 2)
        ).reshape(P, NI * DL)

    in_maps = []
    for c in range(N_CORES):
        pair, hg = c // 4, c % 4
        wo_h = np.ascontiguousarray(
            wo[DL * hg : DL * (hg + 1), :].astype(bf).reshape(HL, P, D).transpose(1, 0, 2)
        ).reshape(P, HL * D)
        in_maps.append(
            {
                "xt": xts[pair],
                "wq": colw(wq, hg),
                "wk": colw(wk, hg),
                "wv": colw(wv, hg),
                "wo": wo_h,
            }
        )
    return in_maps


def _run(inputs, trace=False):
    from concourse.bass_utils import run_bass_kernel_spmd

    in_maps = _host_prep(inputs)
    b_out = np.asarray(inputs["b_out"], dtype=np.float32)

    nc = _get_nc()
    res = run_bass_kernel_spmd(nc, in_maps, core_ids=list(range(N_CORES)), trace=trace)

    y = np.zeros((2, SL, D), dtype=np.float32)
    for c in range(N_CORES):
        y[c // 4] += res.results[c]["y"].astype(np.float32)
    y += b_out[None, None, :]
    out = y.reshape(4, S, D)
    return out, res


def kernel(**inputs) -> np.ndarray:
    out, _ = _run(inputs, trace=False)
    return out
